# revision 21
# baseline (speedup 1.0000x reference)
"""Multi-head attention TRN2 kernel (8 NeuronCores).

Sharding: batch (2) x head-group (4) data/tensor parallel -> 8 cores.
Core c handles batch b = c // 4 and heads [4g, 4g+4) where g = c % 4
(E-dim slice Dg = [256*g, 256*g+256)).

Mask compaction (host): the random 0/1 mask kills whole query rows and
key rows identically (m2 = outer(mask, mask)), and masked-query outputs
are reconstructed on the host (uniform-attention value). So the host
gathers only the unmasked rows of x / kv / attn_bias (~1024 of 2048),
pads to a multiple of 512 (NP), and the device computes attention over
NP x NP only — half the matmul/exp/mul work of the dense version.
Padded queries get eb=1 (finite denominators), padded keys get eb=0
(contribute nothing); padded outputs are discarded on the host.

Device computes, per core, bf16 matmuls with f32 PSUM accumulation:
  QT = (WQ[Dg]/8) @ xc.T         [256, NP]   (1/8 = 1/sqrt(DK))
  KT = WK[Dg] @ kvc.T            [256, NP]
  V  = kvc @ WV[Dg].T            [NP, 256]  (+ ones column per head)
  ST = KT_h.T-blocks @ QT_h      [k, q] scores, transposed
  e  = exp(ST) * ebT             ebT = exp(attn_bias.T) compacted (host)
  U' = [V_h | 1].T @ e           rows 0..63 = unnorm. head out.T, row 64 = denom
  UN = U'[0:64] / denom          (reciprocal_approx_fast on DVE, then a
                                  rank-1 bf16 PE matmul broadcasts 1/den
                                  across the DK partitions via PSUM)
  out_partial = UN.T-blocks @ WO[:, Dg].T   [NP, 1024]

Everything is one fused stream. All DMA issue rides the single Sync
HWDGE queue (its packets stripe across all 16 DMA engines at ~340GB/s;
a second HWDGE queue deadlock-waits on the rotating DMA semaphore pool
and GpSimd's software DGE moves only ~23GB/s), ordered so the bytes
that gate the exp stream (kv0, wk, x0, wq) flow first. kv slices >=1
and the first eb tile prefetch at warm-up end; eb tiles prefetch one
kt2-pair ahead. Each q-block sweeps the key tiles twice, two heads per
sweep, so PSUM fits pj(2) + scores(2x2) + U'(2) = 8 banks; A*V matmuls
trail the score matmuls by 4 key tiles in the PE queue so the
strict-FIFO engine always has independent work ahead of the exp->mul
dependency chain; WO output blocks are emitted mid-next-sweep (kt2p =
NKT/2, NKT-2) so they never block scores while waiting on the
normalize; the final q-block's output blocks ride the freed scores
PSUM ring with alternating ACT/DVE evictions so the tail pipeline
stays 2-deep.

Host: compacts/shards/transposes inputs (bf16), folds scale+mask+exp(bias);
afterwards sums the 4 row-parallel WO partials per batch, adds WO_b,
scatters rows back to the full output, and fills masked-query rows with the
uniform-attention value (reference semantics for fully-masked score rows).
"""

import math
from contextlib import ExitStack

import ml_dtypes
import numpy as np

import concourse.bass as bass
import concourse.bacc as bacc
import concourse.tile as tile
from concourse import mybir
from concourse.bass_utils import run_bass_kernel_spmd

F32 = mybir.dt.float32
F32R = mybir.dt.float32r
BF16 = mybir.dt.bfloat16
AF = mybir.ActivationFunctionType

B, S, E, H, DK = 2, 2048, 1024, 16, 64
NC = 8
DG = 256          # dims per core (4 heads x 64)
HPC = 4           # heads per core
P = 128
QB = 512          # q block
NET = E // P      # 8 contraction tiles over E

TRACE = False
LAST_RESULTS = {}

_NC_CACHE = {}


def _build(NP):
    NKT = NP // P      # key tiles
    NQB = NP // QB     # q blocks
    NKB = NP // QB     # kv slices (512 keys each)

    nc = bacc.Bacc("TRN2", target_bir_lowering=False, debug=False, num_devices=NC)
    # tiled [qb, P, et, QB]: per-partition lines are contiguous
    xT = nc.dram_tensor("xT", [NQB, P, NET, QB], BF16, kind="ExternalInput").ap()
    kvT = nc.dram_tensor("kvT", [NKB, P, NET, QB], BF16, kind="ExternalInput").ap()
    wqT = nc.dram_tensor("wqT", [E, DG], BF16, kind="ExternalInput").ap()
    wkT = nc.dram_tensor("wkT", [E, DG], BF16, kind="ExternalInput").ap()
    wvT = nc.dram_tensor("wvT", [E, DG], BF16, kind="ExternalInput").ap()
    bq = nc.dram_tensor("bq", [DG], F32, kind="ExternalInput")
    bk = nc.dram_tensor("bk", [DG], F32, kind="ExternalInput")
    bv = nc.dram_tensor("bv", [DG], F32, kind="ExternalInput")
    # tiled [qb, P, kt, QB]: per-partition lines are contiguous
    ebT = nc.dram_tensor("ebT", [NQB, P, NKT, QB], BF16, kind="ExternalInput").ap()
    r = nc.dram_tensor("r", [DG, E], BF16, kind="ExternalInput").ap()
    onesdk = nc.dram_tensor("onesdk", [DK], BF16, kind="ExternalInput")
    # tiled [qt2, eb, P, QB]; host reassembles (bf16 partials: the host
    # sums the 4 row-parallel partials in f64 anyway, and f32 would double
    # the tail DMA drain)
    out = nc.dram_tensor("out", [NP // P, 2, P, QB], BF16, kind="ExternalOutput").ap()

    with tile.TileContext(nc) as tc, ExitStack() as ctx:
        const = ctx.enter_context(tc.tile_pool(name="const", bufs=1))

        wq_sb = const.tile([P, NET, DG], BF16, name="wq_sb")
        wk_sb = const.tile([P, NET, DG], BF16, name="wk_sb")
        wv_sb = const.tile([P, NET, DG], BF16, name="wv_sb")

        bq_sb = const.tile([P, 2], F32, name="bq_sb")
        bk_sb = const.tile([P, 2], F32, name="bk_sb")
        bvb_sb = const.tile([P, DG], F32, name="bvb_sb")
        odk_sb = const.tile([1, DK], BF16, name="odk_sb")
        dum_sb = const.tile([1, QB], BF16, name="dum_sb")
        r_sb = const.tile([P, 2, E], BF16, name="r_sb")

        qt_sb = const.tile([P, 2, NP], BF16, name="qt_sb")
        kt_sb = const.tile([P, 2, NP], BF16, name="kt_sb")
        vp_sb = const.tile([P, NKT, HPC, DK + 1], BF16, name="vp_sb")
        un_sb = const.tile([P, 2, NP], BF16, name="un_sb")

        def _load_consts():
            # all DMA issue rides the single Sync HWDGE queue: a second
            # HWDGE queue (Act) deadlock-waits ~27us on the rotating DMA
            # semaphore pool, and GpSimd's software DGE moves only ~23GB/s
            nc.sync.dma_start(
                out=bq_sb, in_=bq.ap().rearrange("(t p) -> p t", p=P)
            )
            nc.sync.dma_start(
                out=bk_sb, in_=bk.ap().rearrange("(t p) -> p t", p=P)
            )
            # bv broadcast over partitions: [P, DG]
            nc.sync.dma_start(
                out=bvb_sb,
                in_=bass.AP(tensor=bv, offset=0, ap=[[0, P], [1, DG]]),
            )
            nc.sync.dma_start(
                out=odk_sb, in_=bass.AP(tensor=onesdk, offset=0, ap=[[0, 1], [1, DK]])
            )
            # init only the denominator-accumulator column of V' to ones
            # (cols 0..DK-1 are fully overwritten by v_pass adds) — via
            # GpSimd memset, NOT a strided DMA: a stride-65 2-byte-element
            # DMA shreds into 4096 minimum-size packets and occupies the
            # DMA pipeline for ~28us, starving everything behind it.
            nc.gpsimd.memset(vp_sb[:, :, :, DK : DK + 1], 1.0)

        # ---- Fused projection/attention/output stream ----
        with tc.tile_pool(name="xk", bufs=2 * max(NKB, NQB)) as xkpool, tc.tile_pool(
            name="ebp", bufs=2
        ) as ebpool, tc.tile_pool(name="fpf", bufs=3) as ffpool, tc.tile_pool(name="fpe", bufs=10) as fpool, tc.tile_pool(
            name="dnu", bufs=2
        ) as dupool, tc.tile_pool(name="dns", bufs=1) as dpool, tc.tile_pool(name="osb", bufs=4) as opool, tc.tile_pool(
            name="pj_ps", bufs=1, space="PSUM"
        ) as pj, tc.tile_pool(name="s_ps", bufs=2, space="PSUM") as sps, tc.tile_pool(
            name="u_ps", bufs=1, space="PSUM"
        ) as ups:
            kv_tiles, x_tiles = {}, {}

            def load_slice(which, j):
                src_t = xT if which == "x" else kvT
                tag = "xt" if which == "x" else "kvt"
                tiles = []
                for eg in range(2):
                    t = xkpool.tile([P, 4, QB], BF16, tag=tag)
                    nc.sync.dma_start(
                        out=t, in_=src_t[j, :, eg * 4 : (eg + 1) * 4]
                    )
                    tiles.append(t)
                return tiles

            def kq_pass(j, w_sb, b_sb, dst, tiles):
                js = slice(j * QB, (j + 1) * QB)
                p2 = pj.tile([P, 2, QB], F32, tag="pj", name="pjk")
                for et in range(NET):
                    for d in range(2):
                        nc.tensor.matmul(
                            p2[:, d], w_sb[:, et, d * P : (d + 1) * P],
                            tiles[et // 4][:, et % 4],
                            start=(et == 0), stop=(et == NET - 1),
                        )
                for d in range(2):
                    nc.vector.tensor_scalar_add(
                        dst[:, d, js], p2[:, d], b_sb[:, d : d + 1]
                    )

            def v_pass(j):
                # two half-passes of 2 key-blocks each so every matmul
                # output starts bank-aligned in the [P,2,QB] pj tile
                kvt = kv_tiles[j]
                for half in range(2):
                    p2 = pj.tile([P, 2, QB], F32, tag="pj", name="pjv")
                    for et in range(NET):
                        for kb2 in range(2):
                            kb = half * 2 + kb2
                            nc.tensor.matmul(
                                p2[:, kb2, 0:DG],
                                kvt[et // 4][:, et % 4, kb * P : (kb + 1) * P],
                                wv_sb[:, et, :],
                                start=(et == 0), stop=(et == NET - 1),
                            )
                    for kb2 in range(2):
                        ktc = j * 4 + half * 2 + kb2
                        nc.vector.tensor_add(
                            vp_sb[:, ktc, :, 0:DK],
                            p2[:, kb2, 0:DG].rearrange("p (h d) -> p h d", h=HPC),
                            bvb_sb.rearrange("p (h d) -> p h d", h=HPC),
                        )

            def load_eb(qb, kt2p, eb_tiles):
                eb2 = ebpool.tile([P, 2, QB], BF16, tag=f"ebt{kt2p}")
                nc.sync.dma_start(out=eb2, in_=ebT[qb, :, kt2p : kt2p + 2])
                eb_tiles[kt2p] = eb2

            def attn_pair(qb, hh, kt2p, ps_u2, eb_tiles, pend):
                qs = slice(qb * QB, (qb + 1) * QB)
                if hh == 0 and kt2p + 2 < NKT:
                    # prefetch the NEXT pair's eb one iteration ahead
                    load_eb(qb, kt2p + 2, eb_tiles)
                eb2 = eb_tiles[kt2p]
                for kt2 in (kt2p, kt2p + 1):
                    ebt = eb2[:, kt2 - kt2p]
                    ks = slice(kt2 * P, (kt2 + 1) * P)
                    ps_s2 = sps.tile([P, 2, QB], F32, tag="pss", name="pss")
                    for jj in range(2):
                        h = hh * 2 + jj
                        d, po = h // 2, (h % 2) * DK
                        nc.tensor.matmul(
                            ps_s2[:, jj], kt_sb[po : po + DK, d, ks],
                            qt_sb[po : po + DK, d, qs], start=True, stop=True,
                        )
                    f2 = ffpool.tile([P, 2, QB], BF16, tag="f", name="f2")
                    nc.scalar.activation(f2, ps_s2, AF.Exp)
                    e2 = fpool.tile([P, 2, QB], BF16, tag="e", name="e2")
                    for jj in range(2):
                        nc.vector.tensor_mul(e2[:, jj], f2[:, jj], ebt)
                    # AV emitted late: the PE queue keeps independent
                    # score matmuls ahead of the exp->mul dependency chain
                    if len(pend) >= 4:
                        pkt, pe2 = pend.pop(0)
                        for jj in range(2):
                            h = hh * 2 + jj
                            nc.tensor.matmul(
                                ps_u2[jj], vp_sb[:, pkt, h, :], pe2[:, jj],
                                start=(pkt == 0), stop=False,
                            )
                    pend.append((kt2, e2))

            def finish_half(qb, hh, ps_u2):
                # normalize U' -> un for heads hh*2, hh*2+1. ACT/DVE copies
                # free the PSUM banks fast; reciprocal_approx_fast needs
                # partition-0 operands (its custom uop mishandles nonzero
                # partition offsets), hence the den staging copies. The
                # reciprocal row is then broadcast across the DK partitions
                # by a rank-1 f32r PE matmul (ones[1,DK].T @ rd[1,QB]) into
                # a borrowed pj-tag PSUM tile — ~0.2us on the PE instead of
                # ~1.1us on GpSimd, and the PE is idle at exactly the
                # boundary points where this chain is critical.
                qs0 = slice(qb * QB, (qb + 1) * QB)
                uraws, dens, rds = [], [], []
                for jj in range(2):
                    h = hh * 2 + jj
                    u_raw = dupool.tile([DK + 1, QB], F32, tag=f"uraw{h}", name="u_raw")
                    # split across ACT/DVE so neither FIFO delays the next
                    # sweep's exps / e-multiplies behind the eviction
                    if jj == 0:
                        nc.scalar.copy(u_raw, ps_u2[jj])
                    else:
                        nc.vector.tensor_copy(u_raw, ps_u2[jj])
                    uraws.append(u_raw)
                for jj in range(2):
                    h = hh * 2 + jj
                    den = dpool.tile([1, QB], F32, tag=f"den{h}", name="den")
                    if jj == 0:
                        nc.scalar.copy(den, uraws[jj][DK : DK + 1, :])
                    else:
                        nc.vector.tensor_copy(den, uraws[jj][DK : DK + 1, :])
                    dens.append(den)
                for jj in range(2):
                    h = hh * 2 + jj
                    rd = dpool.tile([1, QB], F32, tag=f"rd{h}", name="rd")
                    nc.vector.reciprocal_approx_fast(rd, dens[jj])
                    # bf16 staging row: the broadcast matmul wants 2-byte
                    # operands (fp32 would run at 4 cycles/row on the PE)
                    rd16 = dpool.tile([1, QB], BF16, tag=f"rd16{h}", name="rd16")
                    if jj == 0:
                        nc.scalar.copy(rd16, rd)
                    else:
                        nc.vector.tensor_copy(rd16, rd)
                    rds.append(rd16)
                rdb_t = pj.tile([P, 2, QB], F32, tag="pj", name="rdb")
                for jj in range(2):
                    nc.tensor.matmul(
                        rdb_t[0:DK, jj], odk_sb, rds[jj], start=True, stop=True,
                    )
                for jj in range(2):
                    h = hh * 2 + jj
                    d, po = h // 2, (h % 2) * DK
                    nc.vector.tensor_mul(
                        un_sb[po : po + DK, d, qs0], uraws[jj][0:DK, :],
                        rdb_t[0:DK, jj],
                    )

            def out_blocks(qb, half=None, final=False):
                qts = range(qb * 4, qb * 4 + 4)
                if half is not None:
                    qts = qts[half * 2 : half * 2 + 2]
                for qi, qt2 in enumerate(qts):
                    rs = slice(qt2 * P, (qt2 + 1) * P)
                    if final:
                        # scores are done: borrow the freed 2-deep scores
                        # PSUM ring so two output tiles pipeline
                        ps_o = sps.tile([P, 2, QB], F32, tag="pss", name="pso")
                    else:
                        ps_o = pj.tile([P, 2, QB], F32, tag="pj", name="pso")
                    osb = opool.tile([P, 2, QB], BF16, tag="osb")
                    for eb in range(2):
                        es = slice(eb * QB, (eb + 1) * QB)
                        for d in range(2):
                            nc.tensor.matmul(
                                ps_o[:, eb], un_sb[:, d, rs], r_sb[:, d, es],
                                start=(d == 0), stop=(d == 1),
                            )
                        if final:
                            # alternate eviction engine and DMA queue so the
                            # tail chain is never serialized on one engine
                            if (qi + eb) % 2 == 0:
                                nc.scalar.copy(osb[:, eb], ps_o[:, eb])
                                nc.sync.dma_start(out=out[qt2, eb], in_=osb[:, eb])
                            else:
                                nc.vector.tensor_copy(osb[:, eb], ps_o[:, eb])
                                nc.sync.dma_start(out=out[qt2, eb], in_=osb[:, eb])
                        else:
                            if eb == 1:
                                nc.vector.tensor_copy(osb[:, eb], ps_o[:, eb])
                            else:
                                nc.scalar.copy(osb[:, eb], ps_o[:, eb])
                            nc.sync.dma_start(out=out[qt2, eb], in_=osb[:, eb])

            # -- warm-up loads, one ordered queue, in need-order: consts
            # (tiny, they gate the warm bias adds), kv0+wk (K warm-up),
            # x0+wq (Q warm-up -> first scores), wv (v_pass(0) at kt2p=0),
            # qb0's first eb pair (first e-mul), kv1 (K1 proj at kt2p=2),
            # r (first output blocks, much later). --
            nc.gpsimd.memset(dum_sb, 0.0)
            _load_consts()
            kv_tiles[0] = load_slice("kv", 0)
            for ep in range(NET // 2):
                nc.sync.dma_start(
                    out=wk_sb[:, 2 * ep : 2 * ep + 2],
                    in_=wkT[2 * ep * P : (2 * ep + 2) * P].rearrange(
                        "(t p) d -> p t d", p=P
                    ),
                )
            x_tiles[0] = load_slice("x", 0)
            for ep in range(NET // 2):
                nc.sync.dma_start(
                    out=wq_sb[:, 2 * ep : 2 * ep + 2],
                    in_=wqT[2 * ep * P : (2 * ep + 2) * P].rearrange(
                        "(t p) d -> p t d", p=P
                    ),
                )
            for ep in range(NET // 2):
                nc.sync.dma_start(
                    out=wv_sb[:, 2 * ep : 2 * ep + 2],
                    in_=wvT[2 * ep * P : (2 * ep + 2) * P].rearrange(
                        "(t p) d -> p t d", p=P
                    ),
                )
            eb_tiles_all = [{} for _ in range(NQB)]
            load_eb(0, 0, eb_tiles_all[0])
            for j in range(1, NKB):
                kv_tiles[j] = load_slice("kv", j)
            # r is only needed by the first output blocks, much later
            nc.sync.dma_start(out=r_sb, in_=r.rearrange("(t p) e -> p t e", p=P))

            # PE p-state priming: ~3us of continuous execution is needed
            # before the Tensor engine reaches full clock. The DMA fill
            # leaves the PE idle for the first ~13us, so the first real
            # matmuls would otherwise run the whole warm-up at half clock.
            # A dozen throwaway matmuls on a memset tile (no DMA needed)
            # ramp the clock while the fill is still in flight.
            dum_ps = pj.tile([P, 2, QB], F32, tag="pj", name="dum_ps")
            for _ in range(12):
                nc.tensor.matmul(
                    dum_ps[:, 0], dum_sb[0:1, 0:P], dum_sb[0:1, :],
                    start=True, stop=True, skip_group_check=True,
                )

            # Interleaved K/Q warm-up in DMA-arrival order (Q borrows the
            # scores ring so both accumulators are live): scores/exp start
            # as soon as slice-0 data lands. AV matmuls wait on vp via
            # region deps until v_pass lands.
            pk0 = pj.tile([P, 2, QB], F32, tag="pj", name="pjk")
            pq0 = sps.tile([P, 2, QB], F32, tag="pss", name="pjq0")
            for w_sb, tiles, acc in (
                (wk_sb, kv_tiles[0], pk0), (wq_sb, x_tiles[0], pq0),
            ):
                for et in range(NET):
                    for d in range(2):
                        nc.tensor.matmul(
                            acc[:, d], w_sb[:, et, d * P : (d + 1) * P],
                            tiles[et // 4][:, et % 4],
                            start=(et == 0), stop=(et == NET - 1),
                        )
            for d in range(2):
                nc.vector.tensor_scalar_add(
                    kt_sb[:, d, 0:QB], pk0[:, d], bk_sb[:, d : d + 1]
                )
                nc.vector.tensor_scalar_add(
                    qt_sb[:, d, 0:QB], pq0[:, d], bq_sb[:, d : d + 1]
                )

            prev_flush = [None]

            def make_flush(qb, hh, pend, ps_u2):
                def _flush():
                    for pkt, pe2 in pend:
                        for jj in range(2):
                            h = hh * 2 + jj
                            nc.tensor.matmul(
                                ps_u2[jj], vp_sb[:, pkt, h, :], pe2[:, jj],
                                start=(pkt == 0), stop=(pkt == NKT - 1),
                            )
                    finish_half(qb, hh, ps_u2)
                return _flush

            for qb in range(NQB):
                eb_tiles = eb_tiles_all[qb]
                for hh in range(2):
                    ps_u2 = [
                        ups.tile([DK + 1, QB], F32, tag=f"psu{jj}", name=f"psu{jj}")
                        for jj in range(2)
                    ]
                    pend = []
                    for kt2p in range(0, NKT, 2):
                        attn_pair(qb, hh, kt2p, ps_u2, eb_tiles, pend)
                        if kt2p == 0 and prev_flush[0] is not None:
                            # previous half's AV flush + normalize, emitted
                            # AFTER this sweep's first scores so the PE FIFO
                            # has ready work in front of the latency-bound
                            # final AV matmuls
                            prev_flush[0]()
                            prev_flush[0] = None
                        if qb == 0 and hh == 0:
                            # K/V slice projections as sweep filler, placed
                            # just ahead of the first score/AV matmuls that
                            # consume them (K_j before scores kt 4j at
                            # kt2p=4j, V_j before the AV pops): the first
                            # scores are never stuck behind a projection
                            # burst, so the exp stream starts ~7us earlier
                            if kt2p % 4 == 0 and kt2p // 4 < NKB:
                                v_pass(kt2p // 4)
                            if kt2p % 4 == 2 and (kt2p + 2) // 4 < NKB:
                                j = (kt2p + 2) // 4
                                kq_pass(j, wk_sb, bk_sb, kt_sb, kv_tiles[j])
                        if qb < NQB - 1 and hh == 0 and kt2p == 2:
                            x_tiles[qb + 1] = load_slice("x", qb + 1)
                        if qb > 0 and hh == 0 and kt2p == max(NKT // 2, 2):
                            # previous qb's output blocks, two 128-row groups
                            # at a time: PE filler in the back half of the
                            # sweep, well clear of the boundary normalize
                            out_blocks(qb - 1, half=0)
                        if qb > 0 and hh == 0 and kt2p == NKT - 2 and NKT > 4:
                            out_blocks(qb - 1, half=1)
                        if hh == 1 and kt2p == max(NKT - 4, 2) and qb < NQB - 1:
                            kq_pass(qb + 1, wq_sb, bq_sb, qt_sb, x_tiles[qb + 1])
                        if hh == 1 and kt2p == NKT - 2 and qb < NQB - 1:
                            # next qb's first eb pair, ahead of its sweep
                            load_eb(qb + 1, 0, eb_tiles_all[qb + 1])
                    prev_flush[0] = make_flush(qb, hh, list(pend), ps_u2)
            prev_flush[0]()
            out_blocks(NQB - 1, final=True)

    nc.compile()
    return nc


def _get_nc(NP):
    if NP not in _NC_CACHE:
        _NC_CACHE[NP] = _build(NP)
    return _NC_CACHE[NP]


def kernel(x, kv, mask, attn_bias, WQ_w, WQ_b, WK_w, WK_b, WV_w, WV_b, WO_w, WO_b):
    x = np.asarray(x, dtype=np.float32)
    kv = np.asarray(kv, dtype=np.float32)
    mask = np.asarray(mask)
    attn_bias = np.asarray(attn_bias, dtype=np.float32)
    WQ_w = np.asarray(WQ_w, dtype=np.float32)
    WQ_b = np.asarray(WQ_b, dtype=np.float32)
    WK_w = np.asarray(WK_w, dtype=np.float32)
    WK_b = np.asarray(WK_b, dtype=np.float32)
    WV_w = np.asarray(WV_w, dtype=np.float32)
    WV_b = np.asarray(WV_b, dtype=np.float32)
    WO_w = np.asarray(WO_w, dtype=np.float32)
    WO_b = np.asarray(WO_b, dtype=np.float32)

    sc = 1.0 / math.sqrt(DK)
    maskf = mask.astype(np.float32)
    bf = ml_dtypes.bfloat16

    # mask compaction: same index set serves queries and keys (the score
    # mask is outer(mask, mask) and masked-query rows are host-filled)
    idxs = [np.nonzero(mask[b] != 0)[0] for b in range(B)]
    ns = [len(ix) for ix in idxs]
    NP = max(QB, -(-max(ns) // QB) * QB) if max(ns) > 0 else QB
    NQB = NP // QB
    NKT = NP // P

    def _tile_np(aT):
        # [C*P, NP] -> [NQB, P, C, QB]: per-(qb,partition) rows contiguous
        return np.ascontiguousarray(
            aT.reshape(aT.shape[0] // P, P, NQB, QB).transpose(2, 1, 0, 3)
        )

    xTs, kvTs, ebTs = [], [], []
    for b in range(B):
        ix, n = idxs[b], ns[b]
        xc = np.zeros((NP, E), np.float32)
        kvc = np.zeros((NP, E), np.float32)
        eb = np.zeros((NP, NP), np.float32)
        if n:
            xc[:n] = x[b][ix]
            kvc[:n] = kv[b][ix]
            # eb[k', q'] = exp(bias[ix[q'], ix[k']]); padded queries get 1
            # (finite denominators), padded keys get 0 (contribute nothing)
            eb[:n, :n] = np.exp(attn_bias[b][np.ix_(ix, ix)]).T
            eb[:n, n:] = 1.0
        else:
            eb[:, :] = 1.0
        xTs.append(_tile_np(xc.T.astype(bf)))
        kvTs.append(_tile_np(kvc.T.astype(bf)))
        ebTs.append(_tile_np(eb.astype(bf)))

    in_maps = []
    for c in range(NC):
        b, g = c // 4, c % 4
        Dg = slice(DG * g, DG * (g + 1))
        in_maps.append(
            {
                "xT": xTs[b],
                "kvT": kvTs[b],
                "wqT": np.ascontiguousarray((WQ_w[Dg] * sc).T.astype(bf)),
                "wkT": np.ascontiguousarray(WK_w[Dg].T.astype(bf)),
                "wvT": np.ascontiguousarray(WV_w[Dg].T.astype(bf)),
                "bq": np.ascontiguousarray(WQ_b[Dg] * sc),
                "bk": np.ascontiguousarray(WK_b[Dg]),
                "bv": np.ascontiguousarray(WV_b[Dg]),
                "ebT": ebTs[b],
                "r": np.ascontiguousarray(WO_w[:, Dg].T.astype(bf)),
                "onesdk": np.ones(DK, bf),
            }
        )

    nc = _get_nc(NP)
    res = run_bass_kernel_spmd(nc, in_maps, list(range(NC)), trace=TRACE)
    LAST_RESULTS["res"] = res

    out = np.zeros((B, S, E), np.float32)
    for b in range(B):
        acc = np.zeros((NP, E), np.float64)
        for g in range(4):
            ot = res.results[b * 4 + g]["out"]  # [NP//P, 2, P, QB]
            acc += ot.transpose(0, 2, 1, 3).reshape(NP, E).astype(np.float64)
        acc += WO_b.astype(np.float64)[None, :]
        if ns[b]:
            out[b][idxs[b]] = acc[: ns[b]].astype(np.float32)
        # masked-query rows: reference softmax of an all(-1e9) row is uniform
        mrows = maskf[b] == 0.0
        if mrows.any():
            meanV = (
                kv[b].astype(np.float64).mean(axis=0) @ WV_w.astype(np.float64).T
                + WV_b.astype(np.float64)
            )
            mo = meanV @ WO_w.astype(np.float64).T + WO_b.astype(np.float64)
            out[b][mrows, :] = mo[None, :].astype(np.float32)
    return out


# revision 24
# speedup vs baseline: 1.0330x; 1.0330x over previous
"""Multi-head attention TRN2 kernel (8 NeuronCores).

Sharding: batch (2) x head-group (4) data/tensor parallel -> 8 cores.
Core c handles batch b = c // 4 and heads [4g, 4g+4) where g = c % 4
(E-dim slice Dg = [256*g, 256*g+256)).

Mask compaction (host): the random 0/1 mask kills whole query rows and
key rows identically (m2 = outer(mask, mask)), and masked-query outputs
are reconstructed on the host (uniform-attention value). So the host
gathers only the unmasked rows of x / kv / attn_bias (~1024 of 2048),
pads to a multiple of 512 (NP), and the device computes attention over
NP x NP only — half the matmul/exp/mul work of the dense version.
Padded queries get eb=1 (finite denominators), padded keys get eb=0
(contribute nothing); padded outputs are discarded on the host.

Device computes, per core, bf16 matmuls with f32 PSUM accumulation:
  QT = (WQ[Dg]/8) @ xc.T         [256, NP]   (1/8 = 1/sqrt(DK))
  KT = WK[Dg] @ kvc.T            [256, NP]
  V  = kvc @ WV[Dg].T            [NP, 256]  (+ ones column per head)
  ST = KT_h.T-blocks @ QT_h      [k, q] scores, transposed
  e  = exp(ST) * ebT             ebT = exp(attn_bias.T) compacted (host)
  U' = [V_h | 1].T @ e           rows 0..63 = unnorm. head out.T, row 64 = denom
  UN = U'[0:64] / denom          (reciprocal_approx_fast on DVE, then a
                                  rank-1 bf16 PE matmul broadcasts 1/den
                                  across the DK partitions via PSUM)
  out_partial = UN.T-blocks @ WO[:, Dg].T   [NP, 1024]

Everything is one fused stream. All DMA issue rides the single Sync
HWDGE queue (its packets stripe across all 16 DMA engines at ~340GB/s;
a second HWDGE queue deadlock-waits on the rotating DMA semaphore pool
and GpSimd's software DGE moves only ~23GB/s), ordered so the bytes
that gate the exp stream (kv0, wk, x0, wq) flow first. kv slices >=1
and the first eb tile prefetch at warm-up end; eb tiles prefetch one
kt2-pair ahead. Each q-block sweeps the key tiles twice, two heads per
sweep, so PSUM fits pj(2) + scores(2x2) + U'(2) = 8 banks; A*V matmuls
trail the score matmuls by 4 key tiles in the PE queue so the
strict-FIFO engine always has independent work ahead of the exp->mul
dependency chain; WO output blocks are emitted mid-next-sweep (kt2p =
NKT/2, NKT-2) so they never block scores while waiting on the
normalize; the final q-block's output blocks ride the freed scores
PSUM ring with alternating ACT/DVE evictions so the tail pipeline
stays 2-deep.

Host: compacts/shards/transposes inputs (bf16), folds scale+mask+exp(bias);
afterwards sums the 4 row-parallel WO partials per batch, adds WO_b,
scatters rows back to the full output, and fills masked-query rows with the
uniform-attention value (reference semantics for fully-masked score rows).
"""

import math
from contextlib import ExitStack

import ml_dtypes
import numpy as np

import concourse.bass as bass
import concourse.bacc as bacc
import concourse.tile as tile
from concourse import mybir
from concourse.bass_utils import run_bass_kernel_spmd

F32 = mybir.dt.float32
F32R = mybir.dt.float32r
BF16 = mybir.dt.bfloat16
AF = mybir.ActivationFunctionType

B, S, E, H, DK = 2, 2048, 1024, 16, 64
NC = 8
DG = 256          # dims per core (4 heads x 64)
HPC = 4           # heads per core
P = 128
QB = 512          # q block
NET = E // P      # 8 contraction tiles over E

TRACE = False
LAST_RESULTS = {}

_NC_CACHE = {}


def _build(NP):
    NKT = NP // P      # key tiles
    NQB = NP // QB     # q blocks
    NKB = NP // QB     # kv slices (512 keys each)

    nc = bacc.Bacc("TRN2", target_bir_lowering=False, debug=False, num_devices=NC)
    # tiled [qb, P, et, QB]: per-partition lines are contiguous
    xT = nc.dram_tensor("xT", [NQB, P, NET, QB], BF16, kind="ExternalInput").ap()
    kvT = nc.dram_tensor("kvT", [NKB, P, NET, QB], BF16, kind="ExternalInput").ap()
    wqT = nc.dram_tensor("wqT", [E, DG], BF16, kind="ExternalInput").ap()
    wkT = nc.dram_tensor("wkT", [E, DG], BF16, kind="ExternalInput").ap()
    wvT = nc.dram_tensor("wvT", [E, DG], BF16, kind="ExternalInput").ap()
    bq = nc.dram_tensor("bq", [DG], F32, kind="ExternalInput")
    bk = nc.dram_tensor("bk", [DG], F32, kind="ExternalInput")
    bv = nc.dram_tensor("bv", [DG], F32, kind="ExternalInput")
    # tiled [qb, P, kt, QB]: per-partition lines are contiguous
    ebT = nc.dram_tensor("ebT", [NQB, P, NKT, QB], BF16, kind="ExternalInput").ap()
    r = nc.dram_tensor("r", [DG, E], BF16, kind="ExternalInput").ap()
    onesdk = nc.dram_tensor("onesdk", [DK], BF16, kind="ExternalInput")
    # tiled [qt2, eb, P, QB]; host reassembles (bf16 partials: the host
    # sums the 4 row-parallel partials in f64 anyway, and f32 would double
    # the tail DMA drain)
    out = nc.dram_tensor("out", [NP // P, 2, P, QB], BF16, kind="ExternalOutput").ap()

    with tile.TileContext(nc) as tc, ExitStack() as ctx:
        const = ctx.enter_context(tc.tile_pool(name="const", bufs=1))

        wq_sb = const.tile([P, NET, DG], BF16, name="wq_sb")
        wk_sb = const.tile([P, NET, DG], BF16, name="wk_sb")
        wv_sb = const.tile([P, NET, DG], BF16, name="wv_sb")

        bq_sb = const.tile([P, 2], F32, name="bq_sb")
        bk_sb = const.tile([P, 2], F32, name="bk_sb")
        bvb_sb = const.tile([P, DG], F32, name="bvb_sb")
        odk_sb = const.tile([1, DK], BF16, name="odk_sb")
        dum_sb = const.tile([1, QB], BF16, name="dum_sb")
        r_sb = const.tile([P, 2, E], BF16, name="r_sb")

        qt_sb = const.tile([P, 2, NP], BF16, name="qt_sb")
        kt_sb = const.tile([P, 2, NP], BF16, name="kt_sb")
        vp_sb = const.tile([P, NKT, HPC, DK + 1], BF16, name="vp_sb")
        un_sb = const.tile([P, 2, NP], BF16, name="un_sb")

        def _load_consts():
            # all DMA issue rides the single Sync HWDGE queue: a second
            # HWDGE queue (Act) deadlock-waits ~27us on the rotating DMA
            # semaphore pool, and GpSimd's software DGE moves only ~23GB/s
            nc.sync.dma_start(
                out=bq_sb, in_=bq.ap().rearrange("(t p) -> p t", p=P)
            )
            nc.sync.dma_start(
                out=bk_sb, in_=bk.ap().rearrange("(t p) -> p t", p=P)
            )
            # bv broadcast over partitions: [P, DG]
            nc.sync.dma_start(
                out=bvb_sb,
                in_=bass.AP(tensor=bv, offset=0, ap=[[0, P], [1, DG]]),
            )
            nc.sync.dma_start(
                out=odk_sb, in_=bass.AP(tensor=onesdk, offset=0, ap=[[0, 1], [1, DK]])
            )
            # init only the denominator-accumulator column of V' to ones
            # (cols 0..DK-1 are fully overwritten by v_pass adds) — via
            # GpSimd memset, NOT a strided DMA: a stride-65 2-byte-element
            # DMA shreds into 4096 minimum-size packets and occupies the
            # DMA pipeline for ~28us, starving everything behind it.
            nc.gpsimd.memset(vp_sb[:, :, :, DK : DK + 1], 1.0)

        # ---- Fused projection/attention/output stream ----
        with tc.tile_pool(name="xk", bufs=2 * max(NKB, NQB)) as xkpool, tc.tile_pool(
            name="ebp", bufs=2
        ) as ebpool, tc.tile_pool(name="fpf", bufs=3) as ffpool, tc.tile_pool(name="fpe", bufs=10) as fpool, tc.tile_pool(
            name="dnu", bufs=2
        ) as dupool, tc.tile_pool(name="dns", bufs=1) as dpool, tc.tile_pool(name="osb", bufs=4) as opool, tc.tile_pool(
            name="pj_ps", bufs=1, space="PSUM"
        ) as pj, tc.tile_pool(name="s_ps", bufs=2, space="PSUM") as sps, tc.tile_pool(
            name="u_ps", bufs=1, space="PSUM"
        ) as ups:
            kv_tiles, x_tiles = {}, {}

            def load_slice(which, j):
                src_t = xT if which == "x" else kvT
                tag = "xt" if which == "x" else "kvt"
                tiles = []
                for eg in range(2):
                    t = xkpool.tile([P, 4, QB], BF16, tag=tag)
                    nc.sync.dma_start(
                        out=t, in_=src_t[j, :, eg * 4 : (eg + 1) * 4]
                    )
                    tiles.append(t)
                return tiles

            def kq_pass(j, w_sb, b_sb, dst, tiles):
                js = slice(j * QB, (j + 1) * QB)
                p2 = pj.tile([P, 2, QB], F32, tag="pj", name="pjk")
                for et in range(NET):
                    for d in range(2):
                        nc.tensor.matmul(
                            p2[:, d], w_sb[:, et, d * P : (d + 1) * P],
                            tiles[et // 4][:, et % 4],
                            start=(et == 0), stop=(et == NET - 1),
                        )
                for d in range(2):
                    nc.vector.tensor_scalar_add(
                        dst[:, d, js], p2[:, d], b_sb[:, d : d + 1]
                    )

            def v_pass(j):
                # two half-passes of 2 key-blocks each so every matmul
                # output starts bank-aligned in the [P,2,QB] pj tile
                kvt = kv_tiles[j]
                for half in range(2):
                    p2 = pj.tile([P, 2, QB], F32, tag="pj", name="pjv")
                    for et in range(NET):
                        for kb2 in range(2):
                            kb = half * 2 + kb2
                            nc.tensor.matmul(
                                p2[:, kb2, 0:DG],
                                kvt[et // 4][:, et % 4, kb * P : (kb + 1) * P],
                                wv_sb[:, et, :],
                                start=(et == 0), stop=(et == NET - 1),
                            )
                    for kb2 in range(2):
                        ktc = j * 4 + half * 2 + kb2
                        nc.vector.tensor_add(
                            vp_sb[:, ktc, :, 0:DK],
                            p2[:, kb2, 0:DG].rearrange("p (h d) -> p h d", h=HPC),
                            bvb_sb.rearrange("p (h d) -> p h d", h=HPC),
                        )

            def load_eb(qb, kt2p, eb_tiles):
                eb2 = ebpool.tile([P, 2, QB], BF16, tag=f"ebt{kt2p}")
                nc.sync.dma_start(out=eb2, in_=ebT[qb, :, kt2p : kt2p + 2])
                eb_tiles[kt2p] = eb2

            def attn_pair(qb, hh, kt2p, ps_u2, eb_tiles, pend):
                qs = slice(qb * QB, (qb + 1) * QB)
                if hh == 0 and kt2p + 2 < NKT:
                    # prefetch the NEXT pair's eb one iteration ahead
                    load_eb(qb, kt2p + 2, eb_tiles)
                eb2 = eb_tiles[kt2p]
                for kt2 in (kt2p, kt2p + 1):
                    ebt = eb2[:, kt2 - kt2p]
                    ks = slice(kt2 * P, (kt2 + 1) * P)
                    ps_s2 = sps.tile([P, 2, QB], F32, tag="pss", name="pss")
                    for jj in range(2):
                        h = hh * 2 + jj
                        d, po = h // 2, (h % 2) * DK
                        nc.tensor.matmul(
                            ps_s2[:, jj], kt_sb[po : po + DK, d, ks],
                            qt_sb[po : po + DK, d, qs], start=True, stop=True,
                        )
                    f2 = ffpool.tile([P, 2, QB], BF16, tag="f", name="f2")
                    nc.scalar.activation(f2, ps_s2, AF.Exp)
                    e2 = fpool.tile([P, 2, QB], BF16, tag="e", name="e2")
                    for jj in range(2):
                        nc.vector.tensor_mul(e2[:, jj], f2[:, jj], ebt)
                    # AV emitted late: the PE queue keeps independent
                    # score matmuls ahead of the exp->mul dependency chain
                    if len(pend) >= 4:
                        pkt, pe2 = pend.pop(0)
                        for jj in range(2):
                            h = hh * 2 + jj
                            nc.tensor.matmul(
                                ps_u2[jj], vp_sb[:, pkt, h, :], pe2[:, jj],
                                start=(pkt == 0), stop=False,
                            )
                    pend.append((kt2, e2))

            def finish_half(qb, hh, ps_u2):
                # normalize U' -> un for heads hh*2, hh*2+1. ACT/DVE copies
                # free the PSUM banks fast; reciprocal_approx_fast needs
                # partition-0 operands (its custom uop mishandles nonzero
                # partition offsets), hence the den staging copies. The
                # reciprocal row is then broadcast across the DK partitions
                # by a rank-1 f32r PE matmul (ones[1,DK].T @ rd[1,QB]) into
                # a borrowed pj-tag PSUM tile — ~0.2us on the PE instead of
                # ~1.1us on GpSimd, and the PE is idle at exactly the
                # boundary points where this chain is critical.
                qs0 = slice(qb * QB, (qb + 1) * QB)
                uraws, dens, rds = [], [], []
                for jj in range(2):
                    h = hh * 2 + jj
                    u_raw = dupool.tile([DK + 1, QB], F32, tag=f"uraw{h}", name="u_raw")
                    # split across ACT/DVE so neither FIFO delays the next
                    # sweep's exps / e-multiplies behind the eviction
                    if jj == 0:
                        nc.scalar.copy(u_raw, ps_u2[jj])
                    else:
                        nc.vector.tensor_copy(u_raw, ps_u2[jj])
                    uraws.append(u_raw)
                for jj in range(2):
                    h = hh * 2 + jj
                    den = dpool.tile([1, QB], F32, tag=f"den{h}", name="den")
                    if jj == 0:
                        nc.scalar.copy(den, uraws[jj][DK : DK + 1, :])
                    else:
                        nc.vector.tensor_copy(den, uraws[jj][DK : DK + 1, :])
                    dens.append(den)
                for jj in range(2):
                    h = hh * 2 + jj
                    rd = dpool.tile([1, QB], F32, tag=f"rd{h}", name="rd")
                    nc.vector.reciprocal_approx_fast(rd, dens[jj])
                    # bf16 staging row: the broadcast matmul wants 2-byte
                    # operands (fp32 would run at 4 cycles/row on the PE)
                    rd16 = dpool.tile([1, QB], BF16, tag=f"rd16{h}", name="rd16")
                    if jj == 0:
                        nc.scalar.copy(rd16, rd)
                    else:
                        nc.vector.tensor_copy(rd16, rd)
                    rds.append(rd16)
                rdb_t = pj.tile([P, 2, QB], F32, tag="pj", name="rdb")
                for jj in range(2):
                    nc.tensor.matmul(
                        rdb_t[0:DK, jj], odk_sb, rds[jj], start=True, stop=True,
                    )
                for jj in range(2):
                    h = hh * 2 + jj
                    d, po = h // 2, (h % 2) * DK
                    nc.vector.tensor_mul(
                        un_sb[po : po + DK, d, qs0], uraws[jj][0:DK, :],
                        rdb_t[0:DK, jj],
                    )

            def out_blocks(qb, half=None, final=False):
                qts = range(qb * 4, qb * 4 + 4)
                if half is not None:
                    qts = qts[half * 2 : half * 2 + 2]
                for qi, qt2 in enumerate(qts):
                    rs = slice(qt2 * P, (qt2 + 1) * P)
                    if final:
                        # scores are done: borrow the freed 2-deep scores
                        # PSUM ring so two output tiles pipeline
                        ps_o = sps.tile([P, 2, QB], F32, tag="pss", name="pso")
                    else:
                        ps_o = pj.tile([P, 2, QB], F32, tag="pj", name="pso")
                    osb = opool.tile([P, 2, QB], BF16, tag="osb")
                    for eb in range(2):
                        es = slice(eb * QB, (eb + 1) * QB)
                        for d in range(2):
                            nc.tensor.matmul(
                                ps_o[:, eb], un_sb[:, d, rs], r_sb[:, d, es],
                                start=(d == 0), stop=(d == 1),
                            )
                        if final:
                            # alternate eviction engine and DMA queue so the
                            # tail chain is never serialized on one engine
                            if (qi + eb) % 2 == 0:
                                nc.scalar.copy(osb[:, eb], ps_o[:, eb])
                                nc.sync.dma_start(out=out[qt2, eb], in_=osb[:, eb])
                            else:
                                nc.vector.tensor_copy(osb[:, eb], ps_o[:, eb])
                                nc.sync.dma_start(out=out[qt2, eb], in_=osb[:, eb])
                        else:
                            if eb == 1:
                                nc.vector.tensor_copy(osb[:, eb], ps_o[:, eb])
                            else:
                                nc.scalar.copy(osb[:, eb], ps_o[:, eb])
                            nc.sync.dma_start(out=out[qt2, eb], in_=osb[:, eb])

            # -- warm-up loads, one ordered queue, in need-order: consts
            # (tiny, they gate the warm bias adds), kv0+wk (K warm-up),
            # x0+wq (Q warm-up -> first scores), wv (v_pass(0) at kt2p=0),
            # qb0's first eb pair (first e-mul), kv1 (K1 proj at kt2p=2),
            # r (first output blocks, much later). --
            nc.gpsimd.memset(dum_sb, 0.0)
            kv_tiles[0] = load_slice("kv", 0)
            for ep in range(NET // 2):
                nc.sync.dma_start(
                    out=wk_sb[:, 2 * ep : 2 * ep + 2],
                    in_=wkT[2 * ep * P : (2 * ep + 2) * P].rearrange(
                        "(t p) d -> p t d", p=P
                    ),
                )
            x_tiles[0] = load_slice("x", 0)
            for ep in range(NET // 2):
                nc.sync.dma_start(
                    out=wq_sb[:, 2 * ep : 2 * ep + 2],
                    in_=wqT[2 * ep * P : (2 * ep + 2) * P].rearrange(
                        "(t p) d -> p t d", p=P
                    ),
                )
            _load_consts()
            for ep in range(NET // 2):
                nc.sync.dma_start(
                    out=wv_sb[:, 2 * ep : 2 * ep + 2],
                    in_=wvT[2 * ep * P : (2 * ep + 2) * P].rearrange(
                        "(t p) d -> p t d", p=P
                    ),
                )
            eb_tiles_all = [{} for _ in range(NQB)]
            load_eb(0, 0, eb_tiles_all[0])
            for j in range(1, NKB):
                kv_tiles[j] = load_slice("kv", j)
            # r is only needed by the first output blocks, much later
            nc.sync.dma_start(out=r_sb, in_=r.rearrange("(t p) e -> p t e", p=P))

            # PE p-state priming: ~3us of continuous execution is needed
            # before the Tensor engine reaches full clock. The DMA fill
            # leaves the PE idle for the first ~13us, so the first real
            # matmuls would otherwise run the whole warm-up at half clock.
            # A dozen throwaway matmuls on a memset tile (no DMA needed)
            # ramp the clock while the fill is still in flight.
            dum_ps = pj.tile([P, 2, QB], F32, tag="pj", name="dum_ps")
            for i in range(14):
                # alternate banks so consecutive dummies pipeline instead
                # of serializing on the PSUM write-after-write
                nc.tensor.matmul(
                    dum_ps[:, i % 2], dum_sb[0:1, 0:P], dum_sb[0:1, :],
                    start=True, stop=True, skip_group_check=True,
                )

            # Interleaved K/Q warm-up in DMA-arrival order (Q borrows the
            # scores ring so both accumulators are live): scores/exp start
            # as soon as slice-0 data lands. AV matmuls wait on vp via
            # region deps until v_pass lands.
            pk0 = pj.tile([P, 2, QB], F32, tag="pj", name="pjk")
            pq0 = sps.tile([P, 2, QB], F32, tag="pss", name="pjq0")
            for w_sb, tiles, acc in (
                (wk_sb, kv_tiles[0], pk0), (wq_sb, x_tiles[0], pq0),
            ):
                for et in range(NET):
                    for d in range(2):
                        nc.tensor.matmul(
                            acc[:, d], w_sb[:, et, d * P : (d + 1) * P],
                            tiles[et // 4][:, et % 4],
                            start=(et == 0), stop=(et == NET - 1),
                        )
            for d in range(2):
                nc.vector.tensor_scalar_add(
                    kt_sb[:, d, 0:QB], pk0[:, d], bk_sb[:, d : d + 1]
                )
                nc.vector.tensor_scalar_add(
                    qt_sb[:, d, 0:QB], pq0[:, d], bq_sb[:, d : d + 1]
                )

            prev_flush = [None]

            def make_flush(qb, hh, pend, ps_u2):
                def _flush():
                    for pkt, pe2 in pend:
                        for jj in range(2):
                            h = hh * 2 + jj
                            nc.tensor.matmul(
                                ps_u2[jj], vp_sb[:, pkt, h, :], pe2[:, jj],
                                start=(pkt == 0), stop=(pkt == NKT - 1),
                            )
                    finish_half(qb, hh, ps_u2)
                return _flush

            for qb in range(NQB):
                eb_tiles = eb_tiles_all[qb]
                for hh in range(2):
                    ps_u2 = [
                        ups.tile([DK + 1, QB], F32, tag=f"psu{jj}", name=f"psu{jj}")
                        for jj in range(2)
                    ]
                    pend = []
                    for kt2p in range(0, NKT, 2):
                        attn_pair(qb, hh, kt2p, ps_u2, eb_tiles, pend)
                        if kt2p == 0 and prev_flush[0] is not None:
                            # previous half's AV flush + normalize, emitted
                            # AFTER this sweep's first scores so the PE FIFO
                            # has ready work in front of the latency-bound
                            # final AV matmuls
                            prev_flush[0]()
                            prev_flush[0] = None
                        if qb == 0 and hh == 0:
                            # K/V slice projections as sweep filler, placed
                            # just ahead of the first score/AV matmuls that
                            # consume them (K_j before scores kt 4j at
                            # kt2p=4j, V_j before the AV pops): the first
                            # scores are never stuck behind a projection
                            # burst, so the exp stream starts ~7us earlier
                            if kt2p % 4 == 0 and kt2p // 4 < NKB:
                                v_pass(kt2p // 4)
                            if kt2p % 4 == 2 and (kt2p + 2) // 4 < NKB:
                                j = (kt2p + 2) // 4
                                kq_pass(j, wk_sb, bk_sb, kt_sb, kv_tiles[j])
                        if qb < NQB - 1 and hh == 0 and kt2p == 2:
                            x_tiles[qb + 1] = load_slice("x", qb + 1)
                        if qb > 0 and hh == 0 and kt2p == max(NKT // 2, 2):
                            # previous qb's output blocks, two 128-row groups
                            # at a time: PE filler in the back half of the
                            # sweep, well clear of the boundary normalize
                            out_blocks(qb - 1, half=0)
                        if qb > 0 and hh == 0 and kt2p == NKT - 2 and NKT > 4:
                            out_blocks(qb - 1, half=1)
                        if hh == 1 and kt2p == max(NKT - 4, 2) and qb < NQB - 1:
                            kq_pass(qb + 1, wq_sb, bq_sb, qt_sb, x_tiles[qb + 1])
                        if hh == 1 and kt2p == NKT - 2 and qb < NQB - 1:
                            # next qb's first eb pair, ahead of its sweep
                            load_eb(qb + 1, 0, eb_tiles_all[qb + 1])
                    prev_flush[0] = make_flush(qb, hh, list(pend), ps_u2)
            prev_flush[0]()
            out_blocks(NQB - 1, final=True)

    nc.compile()
    return nc


def _get_nc(NP):
    if NP not in _NC_CACHE:
        _NC_CACHE[NP] = _build(NP)
    return _NC_CACHE[NP]


def kernel(x, kv, mask, attn_bias, WQ_w, WQ_b, WK_w, WK_b, WV_w, WV_b, WO_w, WO_b):
    x = np.asarray(x, dtype=np.float32)
    kv = np.asarray(kv, dtype=np.float32)
    mask = np.asarray(mask)
    attn_bias = np.asarray(attn_bias, dtype=np.float32)
    WQ_w = np.asarray(WQ_w, dtype=np.float32)
    WQ_b = np.asarray(WQ_b, dtype=np.float32)
    WK_w = np.asarray(WK_w, dtype=np.float32)
    WK_b = np.asarray(WK_b, dtype=np.float32)
    WV_w = np.asarray(WV_w, dtype=np.float32)
    WV_b = np.asarray(WV_b, dtype=np.float32)
    WO_w = np.asarray(WO_w, dtype=np.float32)
    WO_b = np.asarray(WO_b, dtype=np.float32)

    sc = 1.0 / math.sqrt(DK)
    maskf = mask.astype(np.float32)
    bf = ml_dtypes.bfloat16

    # mask compaction: same index set serves queries and keys (the score
    # mask is outer(mask, mask) and masked-query rows are host-filled)
    idxs = [np.nonzero(mask[b] != 0)[0] for b in range(B)]
    ns = [len(ix) for ix in idxs]
    NP = max(QB, -(-max(ns) // QB) * QB) if max(ns) > 0 else QB
    NQB = NP // QB
    NKT = NP // P

    def _tile_np(aT):
        # [C*P, NP] -> [NQB, P, C, QB]: per-(qb,partition) rows contiguous
        return np.ascontiguousarray(
            aT.reshape(aT.shape[0] // P, P, NQB, QB).transpose(2, 1, 0, 3)
        )

    xTs, kvTs, ebTs = [], [], []
    for b in range(B):
        ix, n = idxs[b], ns[b]
        xc = np.zeros((NP, E), np.float32)
        kvc = np.zeros((NP, E), np.float32)
        eb = np.zeros((NP, NP), np.float32)
        if n:
            xc[:n] = x[b][ix]
            kvc[:n] = kv[b][ix]
            # eb[k', q'] = exp(bias[ix[q'], ix[k']]); padded queries get 1
            # (finite denominators), padded keys get 0 (contribute nothing)
            eb[:n, :n] = np.exp(attn_bias[b][np.ix_(ix, ix)]).T
            eb[:n, n:] = 1.0
        else:
            eb[:, :] = 1.0
        xTs.append(_tile_np(xc.T.astype(bf)))
        kvTs.append(_tile_np(kvc.T.astype(bf)))
        ebTs.append(_tile_np(eb.astype(bf)))

    in_maps = []
    for c in range(NC):
        b, g = c // 4, c % 4
        Dg = slice(DG * g, DG * (g + 1))
        in_maps.append(
            {
                "xT": xTs[b],
                "kvT": kvTs[b],
                "wqT": np.ascontiguousarray((WQ_w[Dg] * sc).T.astype(bf)),
                "wkT": np.ascontiguousarray(WK_w[Dg].T.astype(bf)),
                "wvT": np.ascontiguousarray(WV_w[Dg].T.astype(bf)),
                "bq": np.ascontiguousarray(WQ_b[Dg] * sc),
                "bk": np.ascontiguousarray(WK_b[Dg]),
                "bv": np.ascontiguousarray(WV_b[Dg]),
                "ebT": ebTs[b],
                "r": np.ascontiguousarray(WO_w[:, Dg].T.astype(bf)),
                "onesdk": np.ones(DK, bf),
            }
        )

    nc = _get_nc(NP)
    res = run_bass_kernel_spmd(nc, in_maps, list(range(NC)), trace=TRACE)
    LAST_RESULTS["res"] = res

    out = np.zeros((B, S, E), np.float32)
    for b in range(B):
        acc = np.zeros((NP, E), np.float64)
        for g in range(4):
            ot = res.results[b * 4 + g]["out"]  # [NP//P, 2, P, QB]
            acc += ot.transpose(0, 2, 1, 3).reshape(NP, E).astype(np.float64)
        acc += WO_b.astype(np.float64)[None, :]
        if ns[b]:
            out[b][idxs[b]] = acc[: ns[b]].astype(np.float32)
        # masked-query rows: reference softmax of an all(-1e9) row is uniform
        mrows = maskf[b] == 0.0
        if mrows.any():
            meanV = (
                kv[b].astype(np.float64).mean(axis=0) @ WV_w.astype(np.float64).T
                + WV_b.astype(np.float64)
            )
            mo = meanV @ WO_w.astype(np.float64).T + WO_b.astype(np.float64)
            out[b][mrows, :] = mo[None, :].astype(np.float32)
    return out
